# revision 20
# baseline (speedup 1.0000x reference)
"""GATv2 autoencoder (6 GATv2 conv layers) on 8 Trainium2 NeuronCores.

Strategy (dst-sharded edge-parallel):
  - Nodes padded 20000->20480, 2560 per core; per-core nodes are permuted and
    LPT-binned into 20 windows of 128 nodes with balanced in-edge counts.
  - Each core owns all edges whose dst is in its shard.  Edges are grouped by
    dst-window and padded to a uniform tmax tiles of 128 edges per window.
  - Per layer:  Phase A: xl = x@Wl+bl (and xr) per node tile on TensorE;
    xl shard written to HBM and AllGather'd into a full-table xl_full.
    Phase B (per window): dma_gather xl rows by src; one-hot (iota==dst)
    matrices turn scatter-add and xr-expansion into TensorE matmuls that
    accumulate in PSUM across the window's tiles; attention uses the exact
    identity  sum_c att*leaky(s) = 0.6*sum att*s + 0.4*sum att*|s|  where the
    linear term comes from host-folded weight sum-columns and the abs term
    from ScalarE Abs + VectorE tensor_tensor_reduce.  Softmax without max
    subtraction (alpha range is ~[-7,8]); normalization by 0.25/denominator
    folds the head-mean.
  - LayerNorm params and head-mean are folded on the host; ELU / LN / sigmoid
    run per node tile in Phase C, which also transposes the activations into
    the next layer's lhsT layout.
"""
import sys
import numpy as np

sys.path.insert(0, "/opt/trn_rl_repo")

import concourse.bass as bass
import concourse.tile as tile
from concourse import bacc, mybir
from concourse.bass_utils import run_bass_kernel_spmd

F32 = mybir.dt.float32
I16 = mybir.dt.int16
AF = mybir.ActivationFunctionType
ALU = mybir.AluOpType

NCORES = 8
WIN = 128
NW = 20
SH = WIN * NW            # 2560 node slots per core
NPAD = NCORES * SH       # 20480
N_NODES = 20000
N_EDGES = 160000
H = 4
NEG = 0.2
EPS = 1e-6

# (fin, fout) per conv layer; Ch = H*fout
LAYERS = [(16, 128), (128, 128), (128, 128), (128, 64), (64, 32), (32, 16)]
ACTS = ["elu_ln", "elu_ln", "elu_ln", "none", "elu", "sigmoid"]
A_LIN = 0.5 * (1 + NEG)   # 0.6  (coefficient of sum att*s)
B_ABS = 0.5 * (1 - NEG)   # 0.4  (coefficient of sum att*|s|)


def _grow(ch):
    """gather row length in f32 elements: Ch + 4 lin cols, padded to 256B."""
    return ((ch + 4) * 4 + 255) // 256 * 256 // 4


# ----------------------------------------------------------------------------
# host preprocessing
# ----------------------------------------------------------------------------

def _preprocess(edge_index, edge_attr):
    src = np.asarray(edge_index[0]).astype(np.int64)
    dst = np.asarray(edge_index[1]).astype(np.int64)
    ea = np.asarray(edge_attr, np.float32)
    n = N_NODES
    # self loops with fill_value='mean' (mean of incoming edge_attr per dst)
    s = np.zeros((n, ea.shape[1]), np.float32)
    np.add.at(s, dst, ea)
    cnt = np.bincount(dst, minlength=n).astype(np.float32)
    mean_e = s / np.maximum(cnt, 1.0)[:, None]
    loop = np.arange(n, dtype=np.int64)
    src = np.concatenate([src, loop])
    dst = np.concatenate([dst, loop])
    ea = np.concatenate([ea, mean_e], axis=0)

    deg = np.bincount(dst, minlength=n)

    # core k owns real nodes [k*2500, (k+1)*2500); LPT-bin into NW windows of
    # WIN slots balancing in-edge load.
    perm = np.full((NCORES, SH), -1, np.int64)
    for k in range(NCORES):
        nodes = np.arange(k * 2500, (k + 1) * 2500)
        order = nodes[np.argsort(-deg[nodes], kind="stable")]
        fill = np.zeros(NW, np.int64)
        load = np.zeros(NW, np.int64)
        for n_ in order:
            j = -1
            best = None
            for b in range(NW):
                if fill[b] < WIN and (best is None or load[b] < best):
                    best = load[b]
                    j = b
            perm[k, j * WIN + fill[j]] = n_
            fill[j] += 1
            load[j] += deg[n_]

    slot_of = np.full(n, -1, np.int64)
    valid = perm >= 0
    flat_slots = np.arange(NCORES * SH).reshape(NCORES, SH)
    slot_of[perm[valid]] = flat_slots[valid]
    assert (slot_of >= 0).all()

    slot_dst = slot_of[dst]
    e_core = slot_dst // SH
    e_win = (slot_dst % SH) // WIN
    e_dstloc = slot_dst % WIN

    order = np.lexsort((e_win, e_core))
    counts = np.zeros((NCORES, NW), np.int64)
    np.add.at(counts, (e_core, e_win), 1)
    tmax = int(np.ceil(counts.max() / WIN))
    ET = NW * tmax * WIN

    g_src = np.zeros((NCORES, ET), np.int64)
    g_dst = np.full((NCORES, ET), -1, np.int64)
    g_ea = np.zeros((NCORES, ET, ea.shape[1]), np.float32)
    # bucket fill
    pos = np.zeros((NCORES, NW), np.int64)
    srcslot = slot_of[src]
    for i in order:
        k, w = e_core[i], e_win[i]
        o = w * tmax * WIN + pos[k, w]
        g_src[k, o] = srcslot[i]
        g_dst[k, o] = e_dstloc[i]
        g_ea[k, o] = ea[i]
        pos[k, w] += 1
    return perm, g_src, g_dst, g_ea, tmax


def _fold_params(params):
    """LN-fold + att sum-columns.  Returns list of per-layer dicts."""
    out = []
    names = ["conv1", "conv2", "conv3", "conv4", "recon1", "recon2"]
    pre_ln = [None, "ln1", "ln2", "ln3", None, None]
    for i, nm in enumerate(names):
        p = params[nm]
        Wl = np.asarray(p["Wl"], np.float32).copy()
        bl = np.asarray(p["bl"], np.float32).copy()
        Wr = np.asarray(p["Wr"], np.float32).copy()
        br = np.asarray(p["br"], np.float32).copy()
        We = np.asarray(p["We"], np.float32)
        att = np.asarray(p["att"], np.float32)          # [H, C]
        bias = np.asarray(p["bias"], np.float32)
        if pre_ln[i] is not None:
            w = np.asarray(params[pre_ln[i]]["w"], np.float32)
            b = np.asarray(params[pre_ln[i]]["b"], np.float32)
            bl = bl + b @ Wl
            Wl = w[:, None] * Wl
            br = br + b @ Wr
            Wr = w[:, None] * Wr
        fin, Ch = Wl.shape
        C = Ch // H
        # lin sum-columns: col h = W[:, h*C:(h+1)*C] @ att[h]
        def lincols(W):
            return np.stack([W[:, h * C:(h + 1) * C] @ att[h] for h in range(H)], -1)
        Wl_aug = np.concatenate([Wl, lincols(Wl)], 1)          # [fin, Ch+4]
        Wr_aug = np.concatenate([Wr, lincols(Wr)], 1)
        We_aug = np.concatenate([We, lincols(We)], 1)          # [8, Ch+4]
        def brow(b_):
            return np.concatenate(
                [b_, np.array([b_[h * C:(h + 1) * C] @ att[h] for h in range(H)],
                              np.float32)])[None, :]
        out.append(dict(Wl=Wl_aug, bl=brow(bl), Wr=Wr_aug, br=brow(br),
                        We=We_aug, att=att, bias=bias[None, :].copy()))
    return out


def _wrap_idx(idx):
    """int16 gather-index layout: i -> partition i%16 col i//16, copied to all
    eight 16-partition blocks."""
    ncols = len(idx) // 16
    a = idx.reshape(ncols, 16).T.astype(np.int16)    # [16, ncols]
    return np.tile(a, (8, 1))                        # [128, ncols]


# ----------------------------------------------------------------------------
# device program
# ----------------------------------------------------------------------------

def _build_program(tmax, n_layers=6, mode="full"):
    NT = NW * tmax
    ET = NT * WIN
    nc = bacc.Bacc("TRN2", target_bir_lowering=False, debug=False,
                   num_devices=NCORES)

    x0T = nc.dram_tensor("x0T", [16, SH], F32, kind="ExternalInput")
    gidx = nc.dram_tensor("gidx", [128, ET // 16], I16, kind="ExternalInput")
    dstcol = nc.dram_tensor("dstcol", [128, NT], F32, kind="ExternalInput")
    # eattrT rows 0..7: edge attrs; rows 8+4l..11+4l: host lin_e for layer l
    eattrT = nc.dram_tensor("eattrT", [8 + 4 * n_layers, ET], F32,
                            kind="ExternalInput")
    iota_d = nc.dram_tensor("iota", [128, 128], F32, kind="ExternalInput")
    ident_d = nc.dram_tensor("ident", [128, 128], F32, kind="ExternalInput")
    ones_d = nc.dram_tensor("ones1", [1, 128], F32, kind="ExternalInput")
    Wd, bd, att_d, bias_d = [], [], [], []
    for l in range(n_layers):
        fin, fout = LAYERS[l]
        Ch = H * fout
        Wd.append((
            nc.dram_tensor(f"Wl{l}", [fin, Ch + 4], F32, kind="ExternalInput"),
            nc.dram_tensor(f"Wr{l}", [fin, Ch + 4], F32, kind="ExternalInput"),
            nc.dram_tensor(f"We{l}", [8, Ch + 4], F32, kind="ExternalInput")))
        bd.append((
            nc.dram_tensor(f"bl{l}", [1, Ch + 4], F32, kind="ExternalInput"),
            nc.dram_tensor(f"br{l}", [1, Ch + 4], F32, kind="ExternalInput")))
        att_d.append(nc.dram_tensor(f"att{l}", [1, Ch], F32, kind="ExternalInput"))
        bias_d.append(nc.dram_tensor(f"bias{l}", [1, fout], F32, kind="ExternalInput"))

    last = n_layers - 1
    r_out = nc.dram_tensor("r_out", [SH, LAYERS[last][1]], F32, kind="ExternalOutput")

    xls, xlf = [], []
    for l in range(n_layers):
        g = _grow(H * LAYERS[l][1])
        xls.append(nc.dram_tensor(f"xls{l}", [SH, g], F32))
        xlf.append(nc.dram_tensor(
            f"xlf{l}", [NPAD, g], F32,
            addr_space="Local" if mode == "localxlf" else "Shared"))

    CHMAX = 512
    GMAX = _grow(CHMAX)

    from contextlib import ExitStack
    with tile.TileContext(nc) as tc:
        with ExitStack() as _es:
            def _pool(**kw):
                return _es.enter_context(tc.tile_pool(**kw))
            cst = _pool(name="cst", bufs=1)
            lyc = _pool(name="lyc", bufs=1)          # per-layer consts
            pxT = _pool(name="xT", bufs=2)
            pbig = _pool(name="big", bufs=1)
            pxlg = _pool(name="xlg", bufs=3)
            poh = _pool(name="oh", bufs=3)
            pne = _pool(name="ne", bufs=2)
            pm2 = _pool(name="m2", bufs=2)
            pscr = _pool(name="scr", bufs=1)
            psml = _pool(name="sml", bufs=2)
            pcp = _pool(name="cp", bufs=3)
            pout = _pool(name="outp", bufs=2)
            ps_s = _pool(name="ps_s", bufs=2, space="PSUM")
            ps_o = _pool(name="ps_o", bufs=2, space="PSUM")
            ps_d = _pool(name="ps_d", bufs=2, space="PSUM")
            ps_n = _pool(name="ps_n", bufs=1, space="PSUM")
            ps_l = _pool(name="ps_l", bufs=1, space="PSUM")
            # ---- one-time constant loads ----
            iota_sb = cst.tile([128, 128], F32)
            nc.sync.dma_start(iota_sb[:], iota_d[:])
            ident_sb = cst.tile([128, 128], F32)
            nc.sync.dma_start(ident_sb[:], ident_d[:])
            ones_sb = cst.tile([1, 128], F32)
            nc.sync.dma_start(ones_sb[:], ones_d[:])
            gidx_sb = cst.tile([128, ET // 16], I16)
            nc.sync.dma_start(gidx_sb[:], gidx[:])
            dstc_sb = cst.tile([128, NT], F32)
            nc.sync.dma_start(dstc_sb[:], dstcol[:])
            pea = _pool(name="pea", bufs=2)

            eps_sb = cst.tile([128, 1], F32, tag="eps")
            nc.vector.memset(eps_sb[:], float(EPS))
            xr_buf = pbig.tile([128, NW * (CHMAX + 4)], F32, tag="xrbuf")
            s_norm = pbig.tile([128, NW * CHMAX], F32, tag="snorm")

            xT_cur = pxT.tile([128, SH], F32, tag="xT")
            nc.sync.dma_start(xT_cur[:16, :], x0T[:])

            for l in range(n_layers):
                fin, fout = LAYERS[l]
                Ch = H * fout
                C = fout
                AUG = Ch + 4
                GR = _grow(Ch)

                # ---- per-layer consts ----
                Wl_sb = lyc.tile([fin, AUG], F32, tag="Wl")
                nc.sync.dma_start(Wl_sb[:], Wd[l][0][:])
                Wr_sb = lyc.tile([fin, AUG], F32, tag="Wr")
                nc.sync.dma_start(Wr_sb[:], Wd[l][1][:])
                We_sb = lyc.tile([8, AUG], F32, tag="We")
                nc.sync.dma_start(We_sb[:], Wd[l][2][:])
                bl_sb = lyc.tile([1, AUG], F32, tag="bl")
                nc.sync.dma_start(bl_sb[:], bd[l][0][:])
                br_sb = lyc.tile([1, AUG], F32, tag="br")
                nc.sync.dma_start(br_sb[:], bd[l][1][:])
                attr_sb = lyc.tile([1, Ch], F32, tag="attr")
                nc.sync.dma_start(attr_sb[:], att_d[l][:])
                biasr_sb = lyc.tile([1, fout], F32, tag="biasr")
                nc.sync.dma_start(biasr_sb[:], bias_d[l][:])

                # broadcast att*B_ABS and bias across partitions via K=1 matmul
                ps_b = ps_s.tile([128, CHMAX], F32, tag="ps_s")
                nc.tensor.matmul(ps_b[:, :Ch], ones_sb[:], attr_sb[:],
                                 start=True, stop=True)
                attb_sb = lyc.tile([128, Ch], F32, tag="attb")
                nc.scalar.activation(attb_sb[:], ps_b[:, :Ch], AF.Copy,
                                     scale=float(B_ABS))
                ps_b2 = ps_s.tile([128, CHMAX], F32, tag="ps_s")
                nc.tensor.matmul(ps_b2[:, :fout], ones_sb[:], biasr_sb[:],
                                 start=True, stop=True)
                bias_sb = lyc.tile([128, fout], F32, tag="biasb")
                nc.scalar.activation(bias_sb[:], ps_b2[:, :fout], AF.Copy)

                # ---- Phase A: xl/xr per node tile ----
                for nt in range(NW):
                    lx = xT_cur[:fin, nt * WIN:(nt + 1) * WIN]
                    pa = ps_s.tile([128, CHMAX], F32, tag="ps_s")
                    nc.tensor.matmul(pa[:, :Ch], lx, Wl_sb[:, :Ch],
                                     start=True, stop=False)
                    nc.tensor.matmul(pa[:, :Ch], ones_sb[:], bl_sb[:, :Ch],
                                     start=False, stop=True)
                    pa2 = ps_l.tile([128, H], F32, tag="ps_l")
                    nc.tensor.matmul(pa2[:], lx, Wl_sb[:, Ch:AUG],
                                     start=True, stop=False)
                    nc.tensor.matmul(pa2[:], ones_sb[:], bl_sb[:, Ch:AUG],
                                     start=False, stop=True)
                    xl_t = pcp.tile([128, GR], F32, tag="nodecp")
                    nc.scalar.activation(xl_t[:, :Ch], pa[:, :Ch], AF.Copy)
                    nc.scalar.activation(xl_t[:, Ch:AUG], pa2[:], AF.Copy)
                    if GR > AUG:
                        nc.vector.memset(xl_t[:, AUG:GR], 0.0)
                    nc.sync.dma_start(xls[l][nt * WIN:(nt + 1) * WIN, :], xl_t[:])
                    pb = ps_o.tile([128, CHMAX], F32, tag="ps_o")
                    nc.tensor.matmul(pb[:, :Ch], lx, Wr_sb[:, :Ch],
                                     start=True, stop=False)
                    nc.tensor.matmul(pb[:, :Ch], ones_sb[:], br_sb[:, :Ch],
                                     start=False, stop=True)
                    pb2 = ps_d.tile([128, H], F32, tag="ps_d")
                    nc.tensor.matmul(pb2[:], lx, Wr_sb[:, Ch:AUG],
                                     start=True, stop=False)
                    nc.tensor.matmul(pb2[:], ones_sb[:], br_sb[:, Ch:AUG],
                                     start=False, stop=True)
                    nc.vector.tensor_copy(
                        xr_buf[:, nt * (CHMAX + 4):nt * (CHMAX + 4) + Ch],
                        pb[:, :Ch])
                    nc.vector.tensor_copy(
                        xr_buf[:, nt * (CHMAX + 4) + Ch:nt * (CHMAX + 4) + AUG],
                        pb2[:])

                if mode == "noAG":
                    nc.sync.dma_start(xlf[l][0:SH, :], xls[l][:, :])
                else:
                    nc.gpsimd.collective_compute(
                        "AllGather", ALU.bypass,
                        replica_groups=[list(range(NCORES))],
                        ins=[xls[l].ap().opt()], outs=[xlf[l].ap().opt()])

                # ---- Phase B: edges ----
                GCH = 3            # tiles per gather chunk
                for w in range(NW):
                    po = ps_o.tile([128, CHMAX], F32, tag="ps_o")
                    pd = ps_d.tile([128, H], F32, tag="ps_d")
                    ea_win = pea.tile([8, tmax * WIN], F32, tag="eawin")
                    nc.sync.dma_start(ea_win[:],
                                      eattrT[0:8, w * tmax * WIN:(w + 1) * tmax * WIN])
                    le_win = pea.tile([4, tmax * WIN], F32, tag="lewin")
                    nc.sync.dma_start(
                        le_win[:],
                        eattrT[8 + 4 * l:12 + 4 * l,
                               w * tmax * WIN:(w + 1) * tmax * WIN])
                    xl_chunks = []
                    for c0 in range(0, tmax, GCH):
                        cn = min(GCH, tmax - c0)
                        xch = pxlg.tile([128, cn, GR], F32, tag="xlg")
                        i0 = (w * tmax + c0) * WIN // 16
                        if mode == "nogather":
                            nc.vector.memset(xch[:], 0.01)
                        else:
                            nc.gpsimd.dma_gather(
                                xch[:], xlf[l][:, :],
                                gidx_sb[:, i0:i0 + cn * WIN // 16],
                                cn * WIN, cn * WIN, GR)
                        xl_chunks.append((c0, xch))
                    for t in range(tmax):
                        gt = w * tmax + t
                        xch = xl_chunks[t // GCH][1]
                        xv = xch[:, t % GCH, :]
                        ohs = poh.tile([128, 128], F32, tag="oh")
                        nc.vector.tensor_scalar(
                            ohs[:], iota_sb[:], dstc_sb[:, gt:gt + 1], None,
                            ALU.is_equal)
                        pn = ps_n.tile([128, 128], F32, tag="ps_n")
                        nc.tensor.transpose(pn[:], ohs[:], ident_sb[:])
                        ne_sb = pne.tile([128, 128], F32, tag="ne")
                        nc.scalar.activation(ne_sb[:], pn[:], AF.Copy)

                        ps = ps_s.tile([128, CHMAX], F32, tag="ps_s")
                        pl = ps_l.tile([128, H], F32, tag="ps_l")
                        ea_sl = ea_win[:, t * 128:(t + 1) * 128]
                        nc.tensor.matmul(ps[:, :Ch], ea_sl, We_sb[:, :Ch],
                                         start=True, stop=False)
                        nc.tensor.matmul(ps[:, :Ch], ne_sb[:],
                                         xr_buf[:, w * (CHMAX + 4):w * (CHMAX + 4) + Ch],
                                         start=False, stop=False)
                        nc.tensor.matmul(ps[:, :Ch], ident_sb[:],
                                         xv[:, :Ch], start=False, stop=True)
                        # lin columns
                        le_sl = le_win[:, t * 128:(t + 1) * 128]
                        nc.tensor.matmul(pl[:], le_sl, ident_sb[:4, :4],
                                         start=True, stop=False)
                        nc.tensor.matmul(pl[:], ne_sb[:],
                                         xr_buf[:, w * (CHMAX + 4) + Ch:
                                                w * (CHMAX + 4) + AUG],
                                         start=False, stop=False)
                        nc.tensor.matmul(pl[:], ident_sb[:],
                                         xv[:, Ch:AUG], start=False, stop=True)

                        m2 = pm2.tile([128, CHMAX], F32, tag="m2")
                        nc.scalar.activation(m2[:, :Ch], ps[:, :Ch], AF.Abs)
                        lin = psml.tile([128, H], F32, tag="lin")
                        nc.scalar.activation(lin[:], pl[:], AF.Copy,
                                             scale=float(A_LIN))
                        alpha = psml.tile([128, H], F32, tag="alpha")
                        scr = pscr.tile([128, CHMAX], F32, tag="scr")
                        nc.vector.tensor_mul(scr[:, :Ch], m2[:, :Ch],
                                             attb_sb[:, :Ch])
                        red4 = psml.tile([128, H], F32, tag="red4")
                        nc.vector.tensor_reduce(
                            red4[:], scr[:, :Ch].rearrange("p (h c) -> p h c", h=H),
                            axis=mybir.AxisListType.X, op=ALU.add)
                        nc.vector.tensor_add(alpha[:], red4[:], lin[:])
                        wexp = psml.tile([128, H], F32, tag="wexp")
                        nc.scalar.activation(wexp[:], alpha[:], AF.Exp)
                        contrib = pm2.tile([128, CHMAX], F32, tag="contrib")
                        for hh in range(H):
                            nc.vector.tensor_scalar_mul(
                                contrib[:, hh * C:(hh + 1) * C],
                                xv[:, hh * C:(hh + 1) * C],
                                wexp[:, hh:hh + 1])
                        nc.tensor.matmul(po[:, :Ch], ohs[:], contrib[:, :Ch],
                                         start=(t == 0), stop=(t == tmax - 1))
                        nc.tensor.matmul(pd[:], ohs[:], wexp[:],
                                         start=(t == 0), stop=(t == tmax - 1))
                    den4 = psml.tile([128, H], F32, tag="den4")
                    nc.vector.tensor_scalar(den4[:], pd[:], 4.0, 1e-30,
                                            ALU.mult, ALU.add)
                    denr = psml.tile([128, H], F32, tag="denr")
                    nc.vector.reciprocal(denr[:], den4[:])
                    for hh in range(H):
                        nc.scalar.activation(
                            s_norm[:, w * CHMAX + hh * C:w * CHMAX + (hh + 1) * C],
                            po[:, hh * C:(hh + 1) * C], AF.Copy,
                            scale=denr[:, hh:hh + 1])

                # ---- Phase C: head-mean + bias + act (+ transpose to next xT)
                # two sub-passes so ACT table sets (Exp vs Sqrt) don't thrash
                if l < n_layers - 1:
                    xT_next = pxT.tile([128, SH], F32, tag="xT")
                act = ACTS[l]
                d_buf = pbig.tile([128, NW * 128], F32, tag="dbuf")
                ssq_buf = pbig.tile([128, NW], F32, tag="ssqbuf")
                for nt in range(NW):
                    sv = s_norm[:, nt * CHMAX:nt * CHMAX + Ch]
                    sv3 = sv.rearrange("p (h c) -> p c h", h=H)
                    red = pout.tile([128, C], F32, tag="red")
                    nc.vector.tensor_reduce(red[:], sv3, axis=mybir.AxisListType.X,
                                            op=ALU.add)
                    o1 = pout.tile([128, C], F32, tag="o1")
                    nc.vector.tensor_add(o1[:], red[:], bias_sb[:, :C])
                    if act in ("elu_ln", "elu"):
                        t1 = pout.tile([128, C], F32, tag="t1")
                        nc.vector.tensor_scalar_min(t1[:], o1[:], 0.0)
                        u = pout.tile([128, C], F32, tag="u")
                        nc.scalar.activation(u[:], t1[:], AF.Exp)
                        nc.vector.tensor_scalar_add(u[:], u[:], -1.0)
                        o2 = pout.tile([128, C], F32, tag="o2")
                        nc.vector.tensor_tensor(o2[:], o1[:], u[:], ALU.max)
                    elif act == "sigmoid":
                        o2 = pout.tile([128, C], F32, tag="o2")
                        nc.scalar.activation(o2[:], o1[:], AF.Sigmoid)
                    else:
                        o2 = o1
                    if act == "elu_ln":
                        sm = pout.tile([128, 1], F32, tag="sm")
                        nc.vector.reduce_sum(sm[:], o2[:], axis=mybir.AxisListType.X)
                        mu = pout.tile([128, 1], F32, tag="mu")
                        nc.vector.tensor_scalar_mul(mu[:], sm[:], 1.0 / C)
                        dsl = d_buf[:, nt * 128:nt * 128 + C]
                        nc.vector.tensor_scalar(dsl, o2[:], mu[:], None,
                                                ALU.subtract)
                        dsc = pout.tile([128, C], F32, tag="dsc")
                        nc.scalar.activation(dsc[:], dsl, AF.Square)
                        nc.vector.reduce_sum(ssq_buf[:, nt:nt + 1], dsc[:],
                                             axis=mybir.AxisListType.X)
                    else:
                        # stash the finished tile in d_buf for the second pass
                        nc.vector.tensor_copy(d_buf[:, nt * 128:nt * 128 + C],
                                              o2[:])
                if act == "elu_ln":
                    sd = pout.tile([128, NW], F32, tag="sd")
                    nc.scalar.activation(sd[:], ssq_buf[:], AF.Sqrt,
                                         bias=eps_sb[:], scale=1.0 / C)
                    rs = pout.tile([128, NW], F32, tag="rs")
                    nc.vector.reciprocal(rs[:], sd[:])
                for nt in range(NW):
                    dsl = d_buf[:, nt * 128:nt * 128 + C]
                    if act == "elu_ln":
                        fin_t = pout.tile([128, C], F32, tag="fin")
                        nc.vector.tensor_scalar(fin_t[:], dsl,
                                                rs[:, nt:nt + 1], None, ALU.mult)
                    else:
                        fin_t = None
                    src_ap = fin_t[:] if fin_t is not None else dsl
                    if l < n_layers - 1:
                        ptr = ps_n.tile([128, 128], F32, tag="ps_n")
                        nc.tensor.transpose(ptr[:C, :], src_ap, ident_sb[:])
                        nc.scalar.activation(xT_next[:C, nt * WIN:(nt + 1) * WIN],
                                             ptr[:C, :], AF.Copy)
                    else:
                        nc.sync.dma_start(r_out[nt * WIN:(nt + 1) * WIN, :],
                                          src_ap)
                if l < n_layers - 1:
                    xT_cur = xT_next

    nc.compile()
    return nc


_PROG_CACHE = {}


def _get_program(tmax, n_layers=6, mode="full"):
    key = (tmax, n_layers, mode)
    if key not in _PROG_CACHE:
        _PROG_CACHE[key] = _build_program(tmax, n_layers, mode)
    return _PROG_CACHE[key]


# ----------------------------------------------------------------------------
# entry point
# ----------------------------------------------------------------------------

def kernel(node_features, edge_index, edge_attr, params):
    node_features = np.asarray(node_features, np.float32)
    x0 = node_features
    perm, g_src, g_dst, g_ea, tmax = _preprocess(edge_index, edge_attr)
    L = _fold_params(params)
    n_layers = 6

    nc = _get_program(tmax, n_layers)

    NT = NW * tmax
    ET = NT * WIN

    iota = np.tile(np.arange(128, dtype=np.float32)[None, :], (128, 1))
    ident = np.eye(128, dtype=np.float32)
    ones1 = np.ones((1, 128), np.float32)

    in_maps = []
    for k in range(NCORES):
        xs = np.zeros((SH, 16), np.float32)
        m = perm[k] >= 0
        xs[m] = x0[perm[k][m]]
        # eattrT with per-layer lin_e rows
        ea_rows = [g_ea[k].T]                          # [8, ET]
        for l in range(n_layers):
            Ch = H * LAYERS[l][1]
            C = Ch // H
            We = L[l]["We"][:, :Ch]
            att = L[l]["att"]
            lin_e = np.stack([g_ea[k] @ (We[:, h * C:(h + 1) * C] @ att[h])
                              for h in range(H)], 0)   # [4, ET]
            ea_rows.append(lin_e.astype(np.float32))
        im = {
            "x0T": np.ascontiguousarray(xs.T),
            "gidx": _wrap_idx(g_src[k]),
            "dstcol": np.ascontiguousarray(
                g_dst[k].reshape(NT, WIN).T.astype(np.float32)),
            "eattrT": np.ascontiguousarray(np.concatenate(ea_rows, 0)),
            "iota": iota, "ident": ident, "ones1": ones1,
        }
        for l in range(n_layers):
            im[f"Wl{l}"] = L[l]["Wl"]
            im[f"Wr{l}"] = L[l]["Wr"]
            im[f"We{l}"] = L[l]["We"]
            im[f"bl{l}"] = L[l]["bl"]
            im[f"br{l}"] = L[l]["br"]
            im[f"att{l}"] = np.ascontiguousarray(L[l]["att"].reshape(1, -1))
            im[f"bias{l}"] = L[l]["bias"]
        in_maps.append(im)

    res = run_bass_kernel_spmd(nc, in_maps, core_ids=list(range(NCORES)))

    fout_last = LAYERS[n_layers - 1][1]
    r = np.zeros((N_NODES, fout_last), np.float32)
    for k in range(NCORES):
        m = perm[k] >= 0
        r[perm[k][m]] = res.results[k]["r_out"][m]
    return (node_features, r)
